# revision 8
# baseline (speedup 1.0000x reference)
# Bidirectional TreeLSTM (heap-indexed complete binary tree) on 8 trn2 NeuronCores.
#
# Key algorithmic reductions vs the reference:
#   * The output reads only c_bu[:, 0] and c_td[:, 0].  The entire top-down
#     recurrence below the root is dead code; only the root's W_iou_td path is
#     computed.  x = relu(feats @ W_mlp.T + b) is needed only at the 512 leaf
#     nodes (bottom-up) and at node 0 (top-down root), so only 513/1023 of
#     feats is ever loaded.
#   * Sharding: data-parallel over B (32 trees/core), weights replicated.
#
# Layout per core: tokens are columns (node-major, batch-minor); features on
# the 128 partitions.  Levels processed leaves->root depth-first in chunks of
# 512 columns; each chunk's h / f*c go directly into the parent level's
# accumulator tile, and the parent's iou matmuls read even/odd child blocks
# with PSUM accumulation (h_l + h_r folded into the matmul).

import numpy as np

B, DEPTH, X, H = 256, 10, 128, 128
NCOUT = 128
NCORES = 8
BC = B // NCORES  # trees per core
NLEAF = 512
CHUNK = 512

_CACHE = {}
LAST_RESULTS = None  # BassKernelResults from the most recent run (for test.py)


def _t(level):
    """Tokens (columns) at a tree level, per core."""
    return BC * (1 << level)


def _split_multi_waits(nc):
    """This container's walrus supports only ONE embedded sem-wait per
    instruction.  Hoist extra waits onto same-engine NOPs inserted directly
    before each offending instruction (sem-wait order is immaterial)."""
    import concourse.mybir as mybir

    n_split = 0
    for fn in nc.m.functions:
        for bb in fn.blocks:
            out = []
            changed = False
            for inst in bb.instructions:
                si = inst.sync_info
                if si is not None and len(si.on_wait) > 1:
                    waits = list(si.on_wait)
                    for k, wt in enumerate(waits[:-1]):
                        nop = mybir.InstNoOp(
                            name=f"{inst.name}_wsplit{k}", ins=[], outs=[]
                        )
                        nop.engine = inst.engine
                        nop.sync_info = mybir.SyncInfo(on_wait=[wt], on_update=[])
                        out.append(nop)
                        n_split += 1
                    inst.sync_info = mybir.SyncInfo(
                        on_wait=waits[-1:], on_update=list(si.on_update)
                    )
                    changed = True
                out.append(inst)
            if changed:
                bb.instructions = out
    return n_split


def _build_nc():
    from contextlib import ExitStack

    import concourse.bass as bass
    import concourse.mybir as mybir
    import concourse.tile as tile

    fp32 = mybir.dt.float32
    AF = mybir.ActivationFunctionType
    Alu = mybir.AluOpType

    nc = bass.Bass("TRN2", debug=False)

    feats_leafT = nc.dram_tensor(
        "feats_leafT", [X, NLEAF * BC], fp32, kind="ExternalInput"
    ).ap()
    feats_rootT = nc.dram_tensor("feats_rootT", [X, BC], fp32, kind="ExternalInput").ap()
    wbig_d = nc.dram_tensor("wbig", [128, 12 * 128], fp32, kind="ExternalInput").ap()
    bbig_d = nc.dram_tensor("bbig", [128, 8], fp32, kind="ExternalInput").ap()
    out_d = nc.dram_tensor("out", [NCOUT, BC], fp32, kind="ExternalOutput").ap()

    with tile.TileContext(nc) as tc, ExitStack() as ctx:
        const = ctx.enter_context(tc.tile_pool(name="const", bufs=1))
        feats_pool = ctx.enter_context(tc.tile_pool(name="feats", bufs=3))
        gates = ctx.enter_context(tc.tile_pool(name="gates", bufs=3))
        accp = ctx.enter_context(tc.tile_pool(name="acc", bufs=2))
        psum = ctx.enter_context(tc.tile_pool(name="psum", bufs=1, space="PSUM"))

        wbig = const.tile([128, 12 * 128], fp32, name="wbig_sb")
        nc.sync.dma_start(wbig, wbig_d)
        bbig = const.tile([128, 8], fp32, name="bbig_sb")
        nc.sync.dma_start(bbig, bbig_d)

        def W(i):
            return wbig[:, 128 * i : 128 * (i + 1)]

        w_mlp = W(0)
        w_iou = [W(1), W(2), W(3)]  # leaf W_iou_bu.T slices (i, o, u)
        u_iou = [W(4), W(5), W(6)]  # internal U_iou_bu.T slices
        uf = W(7)
        wtd_i, wtd_u = W(8), W(9)
        wfc_bu, wfc_td = W(10), W(11)

        def bias(i):
            return bbig[:, i : i + 1]

        b_mlp, bi, bo, bu, bf, bi_td, bu_td, b_fc = [bias(i) for i in range(8)]

        # parent level -> (h_full, fc_full) accumulator tiles holding the
        # child level's h and f*c for one parent chunk (width 2*parent_chunk).
        acc = {}

        def get_acc(pl):
            if pl not in acc:
                w2 = 2 * min(_t(pl), CHUNK)
                hf = accp.tile([128, w2], fp32, tag=f"hf{pl}", bufs=2, name=f"hf{pl}")
                ff = accp.tile([128, w2], fp32, tag=f"ff{pl}", bufs=2, name=f"ff{pl}")
                acc[pl] = (hf, ff)
            return acc[pl]

        def evenodd(t, w2):
            # [128, w2] -> even/odd node-block views [128, w2/64, 32]
            v = t[:, 0:w2].rearrange("p (n t b) -> p n t b", t=2, b=BC)
            return v[:, :, 0, :], v[:, :, 1, :]

        croot = {}

        def process_chunk(l, idx, src=None):
            """Process one 512-col chunk of level l (gates + f), writing h and
            f*c into the parent accumulator."""
            w = min(_t(l), CHUNK)
            need_h = l > 0
            iou_ps = psum.tile([128, 3, CHUNK], fp32, tag="iou", bufs=2, name="iou_ps")
            gs = [0, 1, 2] if need_h else [0, 2]
            if l == DEPTH - 1:
                for g in gs:
                    nc.tensor.matmul(iou_ps[:, g, :w], w_iou[g], src, start=True, stop=True)
            else:
                hf, ff = acc.pop(l)
                w2 = 2 * w
                he, ho = evenodd(hf, w2)
                for g in gs:
                    nc.tensor.matmul(iou_ps[:, g, :w], u_iou[g], he, start=True, stop=False)
                    nc.tensor.matmul(iou_ps[:, g, :w], u_iou[g], ho, start=False, stop=True)

            si = gates.tile([128, w], fp32, tag="si", bufs=3, name="si")
            nc.scalar.activation(si, iou_ps[:, 0, :w], AF.Sigmoid, bias=bi)
            tu = gates.tile([128, w], fp32, tag="tu", bufs=3, name="tu")
            nc.scalar.activation(tu, iou_ps[:, 2, :w], AF.Tanh, bias=bu)

            if need_h:
                ct = gates.tile([128, w], fp32, tag="c", bufs=3, name="ct")
            else:
                ct = const.tile([128, w], fp32, name="croot_bu")
            nc.vector.tensor_mul(ct, si, tu)
            if l < DEPTH - 1:
                fe, fo = evenodd(ff, w2)
                ctv = ct[:, 0:w].rearrange("p (n b) -> p n b", b=BC)
                nc.vector.tensor_add(ctv, ctv, fe)
                nc.vector.tensor_add(ctv, ctv, fo)

            if not need_h:
                croot["bu"] = ct
                return

            so = gates.tile([128, w], fp32, tag="so", bufs=3, name="so")
            nc.scalar.activation(so, iou_ps[:, 1, :w], AF.Sigmoid, bias=bo)
            tct = gates.tile([128, w], fp32, tag="tc", bufs=3, name="tct")
            nc.scalar.activation(tct, ct, AF.Tanh)

            pl = l - 1
            hfp, ffp = get_acc(pl)
            w2p = 2 * min(_t(pl), CHUNK)
            off = (idx % (w2p // w)) * w
            hslot = hfp[:, off : off + w]
            nc.vector.tensor_mul(hslot, so, tct)

            zf = psum.tile([128, CHUNK], fp32, tag="zf", bufs=1, name="zf")
            nc.tensor.matmul(zf[:, :w], uf, hslot, start=True, stop=True)
            fg = gates.tile([128, w], fp32, tag="f", bufs=3, name="fg")
            nc.scalar.activation(fg, zf[:, :w], AF.Sigmoid, bias=bf)
            nc.gpsimd.tensor_mul(ffp[:, off : off + w], fg, ct)

        def on_complete(l, idx):
            if l == 0:
                return
            pl = l - 1
            w = min(_t(l), CHUNK)
            pw = min(_t(pl), CHUNK)
            cpp = (2 * pw) // w  # child chunks per parent chunk
            if (idx + 1) % cpp == 0:
                pidx = idx // cpp
                process_chunk(pl, pidx)
                on_complete(pl, pidx)

        # Bottom-up pass, depth-first over leaf chunks.
        for j in range(NLEAF * BC // CHUNK):
            ft = feats_pool.tile([128, CHUNK], fp32, tag="feats", bufs=3, name="ft")
            nc.sync.dma_start(ft, feats_leafT[:, j * CHUNK : (j + 1) * CHUNK])
            mlp_ps = psum.tile([128, CHUNK], fp32, tag="mlp", bufs=1, name="mlp_ps")
            nc.tensor.matmul(mlp_ps, w_mlp, ft, start=True, stop=True)
            xt = gates.tile([128, CHUNK], fp32, tag="x", bufs=3, name="xt")
            nc.vector.tensor_scalar(xt, mlp_ps, b_mlp, 0.0, Alu.add, Alu.max)
            process_chunk(DEPTH - 1, j, src=xt)
            on_complete(DEPTH - 1, j)

        # Top-down root: c_td = sigmoid(i)*tanh(u) on node 0 only.
        ftr = feats_pool.tile([128, BC], fp32, tag="feats", bufs=3, name="ftr")
        nc.sync.dma_start(ftr, feats_rootT)
        mlp2 = psum.tile([128, CHUNK], fp32, tag="mlp", bufs=1, name="mlp2")
        nc.tensor.matmul(mlp2[:, :BC], w_mlp, ftr, start=True, stop=True)
        xr = gates.tile([128, BC], fp32, tag="x", bufs=3, name="xr")
        nc.vector.tensor_scalar(xr, mlp2[:, :BC], b_mlp, 0.0, Alu.add, Alu.max)
        iou_td = psum.tile([128, 3, CHUNK], fp32, tag="iou", bufs=2, name="iou_td")
        nc.tensor.matmul(iou_td[:, 0, :BC], wtd_i, xr, start=True, stop=True)
        nc.tensor.matmul(iou_td[:, 2, :BC], wtd_u, xr, start=True, stop=True)
        si_td = gates.tile([128, BC], fp32, tag="si", bufs=3, name="si_td")
        nc.scalar.activation(si_td, iou_td[:, 0, :BC], AF.Sigmoid, bias=bi_td)
        tu_td = gates.tile([128, BC], fp32, tag="tu", bufs=3, name="tu_td")
        nc.scalar.activation(tu_td, iou_td[:, 2, :BC], AF.Tanh, bias=bu_td)
        c_td = const.tile([128, BC], fp32, name="c_td")
        nc.vector.tensor_mul(c_td, si_td, tu_td)

        # Readout: out = W_fc @ [c_bu_root; c_td_root] + b_fc  (output kept
        # transposed as [NC, BC]; host transposes back).
        fc_ps = psum.tile([128, CHUNK], fp32, tag="mlp", bufs=1, name="fc_ps")
        nc.tensor.matmul(fc_ps[:, :BC], wfc_bu, croot["bu"], start=True, stop=False)
        nc.tensor.matmul(fc_ps[:, :BC], wfc_td, c_td, start=False, stop=True)
        out_sb = gates.tile([128, BC], fp32, tag="si", bufs=3, name="out_sb")
        nc.scalar.activation(out_sb, fc_ps[:, :BC], AF.Identity, bias=b_fc)
        nc.sync.dma_start(out_d, out_sb)

    _split_multi_waits(nc)
    return nc


def _prep_shared(inputs):
    f32 = np.float32

    def T(a):
        return np.ascontiguousarray(np.asarray(a, f32).T)

    W_fc = np.asarray(inputs["W_fc"], f32)
    wbig = np.concatenate(
        [
            T(inputs["W_mlp"]),
            T(inputs["W_iou_bu"]),  # [128, 384] = i|o|u
            T(inputs["U_iou_bu"]),
            T(inputs["Uf_bu_w"]),
            T(np.asarray(inputs["W_iou_td"], f32)[0:128, :]),  # i slice
            T(np.asarray(inputs["W_iou_td"], f32)[256:384, :]),  # u slice
            T(W_fc[:, 0:128]),
            T(W_fc[:, 128:256]),
        ],
        axis=1,
    )
    b_iou_bu = np.asarray(inputs["b_iou_bu"], f32)
    b_iou_td = np.asarray(inputs["b_iou_td"], f32)
    bbig = np.stack(
        [
            np.asarray(inputs["b_mlp"], f32),
            b_iou_bu[0:128],
            b_iou_bu[128:256],
            b_iou_bu[256:384],
            np.asarray(inputs["Uf_bu_b"], f32),
            b_iou_td[0:128],
            b_iou_td[256:384],
            np.asarray(inputs["b_fc"], f32),
        ],
        axis=1,
    )
    return np.ascontiguousarray(wbig), np.ascontiguousarray(bbig)


def _get_runner():
    """Build the bass program once and return a cached jitted 8-core runner.

    Mirrors concourse.bass2jax.run_bass_via_pjrt but caches the jitted
    callable so repeated kernel() calls don't re-trace/re-compile."""
    if "runner" in _CACHE:
        return _CACHE["runner"]

    import jax
    import jax.numpy as jnp
    from jax.sharding import Mesh, PartitionSpec
    from jax.experimental.shard_map import shard_map

    import concourse.mybir as mybir
    from concourse import bass2jax

    bass2jax.install_neuronx_cc_hook()
    nc = _build_nc()

    partition_name = (
        nc.partition_id_tensor.name if nc.partition_id_tensor is not None else None
    )
    in_names, out_names, out_avals = [], [], []
    for alloc in nc.m.functions[0].allocations:
        if not isinstance(alloc, mybir.MemoryLocationSet):
            continue
        name = alloc.memorylocations[0].name
        if alloc.kind == "ExternalInput":
            if name != partition_name:
                in_names.append(name)
        elif alloc.kind == "ExternalOutput":
            out_names.append(name)
            out_avals.append(
                jax.core.ShapedArray(
                    tuple(alloc.tensor_shape), mybir.dt.np(alloc.dtype)
                )
            )
    n_params = len(in_names)
    all_in_names = in_names + out_names
    if partition_name is not None:
        all_in_names = all_in_names + [partition_name]

    def _body(*args):
        operands = list(args)
        if partition_name is not None:
            operands.append(bass2jax.partition_id_tensor())
        outs = bass2jax._bass_exec_p.bind(
            *operands,
            out_avals=tuple(out_avals),
            in_names=tuple(all_in_names),
            out_names=tuple(out_names),
            lowering_input_output_aliases=(),
            sim_require_finite=True,
            sim_require_nnan=True,
            nc=nc,
        )
        return tuple(outs)

    devices = jax.devices()[:NCORES]
    mesh = Mesh(np.asarray(devices), ("core",))
    n_outs = len(out_names)
    sharded = jax.jit(
        shard_map(
            _body,
            mesh=mesh,
            in_specs=(PartitionSpec("core"),) * (n_params + n_outs),
            out_specs=(PartitionSpec("core"),) * n_outs,
            check_rep=False,
        ),
        keep_unused=True,
    )

    runner = {
        "nc": nc,
        "sharded": sharded,
        "in_names": in_names,
        "out_names": out_names,
        "out_avals": out_avals,
    }
    _CACHE["runner"] = runner
    return runner


def _run_spmd(in_maps):
    """Execute on 8 cores; returns list of per-core output dicts."""
    r = _get_runner()
    concat_in = [
        np.concatenate([m[name] for m in in_maps], axis=0) for name in r["in_names"]
    ]
    concat_zeros = [
        np.zeros((NCORES * a.shape[0], *a.shape[1:]), a.dtype) for a in r["out_avals"]
    ]
    out_arrs = r["sharded"](*concat_in, *concat_zeros)
    return [
        {
            name: np.asarray(out_arrs[i]).reshape(NCORES, *r["out_avals"][i].shape)[c]
            for i, name in enumerate(r["out_names"])
        }
        for c in range(NCORES)
    ]


def kernel(**inputs):
    global LAST_RESULTS

    feats = np.asarray(inputs["feats"], np.float32)  # [256, 1023, 128]
    wbig, bbig = _prep_shared(inputs)

    in_maps = []
    for c in range(NCORES):
        fb = feats[c * BC : (c + 1) * BC]  # [BC, 1023, 128]
        leafT = np.ascontiguousarray(
            fb[:, NLEAF - 1 : 2 * NLEAF - 1, :].transpose(2, 1, 0).reshape(X, NLEAF * BC)
        )
        rootT = np.ascontiguousarray(fb[:, 0, :].T)
        in_maps.append(
            {
                "feats_leafT": leafT,
                "feats_rootT": rootT,
                "wbig": wbig,
                "bbig": bbig,
            }
        )

    results = _run_spmd(in_maps)
    LAST_RESULTS = results
    out = np.concatenate([results[c]["out"].T for c in range(NCORES)], axis=0)
    return np.ascontiguousarray(out.astype(np.float32))


# revision 9
# speedup vs baseline: 16.8820x; 16.8820x over previous
# Bidirectional TreeLSTM (heap-indexed complete binary tree) on 8 trn2 NeuronCores.
#
# Key algorithmic reductions vs the reference:
#   * The output reads only c_bu[:, 0] and c_td[:, 0].  The entire top-down
#     recurrence below the root is dead code; only the root's W_iou_td path is
#     computed.  x = relu(feats @ W_mlp.T + b) is needed only at the 512 leaf
#     nodes (bottom-up) and at node 0 (top-down root), so only 513/1023 of
#     feats is ever loaded.
#   * Sharding: data-parallel over B (32 trees/core), weights replicated.
#
# Layout per core: tokens are columns (node-major, batch-minor); features on
# the 128 partitions.  Levels processed leaves->root depth-first in chunks of
# 512 columns; each chunk's h / f*c go directly into the parent level's
# accumulator tile, and the parent's iou matmuls read even/odd child blocks
# with PSUM accumulation (h_l + h_r folded into the matmul).

import numpy as np

B, DEPTH, X, H = 256, 10, 128, 128
NCOUT = 128
NCORES = 8
BC = B // NCORES  # trees per core
NLEAF = 512
CHUNK = 512

_CACHE = {}
LAST_RESULTS = None  # BassKernelResults from the most recent run (for test.py)


def _t(level):
    """Tokens (columns) at a tree level, per core."""
    return BC * (1 << level)


def _split_multi_waits(nc):
    """This container's walrus supports only ONE embedded sem-wait per
    instruction.  Hoist extra waits onto same-engine NOPs inserted directly
    before each offending instruction (sem-wait order is immaterial)."""
    import concourse.mybir as mybir

    n_split = 0
    for fn in nc.m.functions:
        for bb in fn.blocks:
            out = []
            changed = False
            for inst in bb.instructions:
                si = inst.sync_info
                if si is not None and len(si.on_wait) > 1:
                    waits = list(si.on_wait)
                    for k, wt in enumerate(waits[:-1]):
                        nop = mybir.InstNoOp(
                            name=f"{inst.name}_wsplit{k}", ins=[], outs=[]
                        )
                        nop.engine = inst.engine
                        nop.sync_info = mybir.SyncInfo(on_wait=[wt], on_update=[])
                        out.append(nop)
                        n_split += 1
                    inst.sync_info = mybir.SyncInfo(
                        on_wait=waits[-1:], on_update=list(si.on_update)
                    )
                    changed = True
                out.append(inst)
            if changed:
                bb.instructions = out
    return n_split


def _build_nc():
    from contextlib import ExitStack

    import concourse.bass as bass
    import concourse.mybir as mybir
    import concourse.tile as tile

    fp32 = mybir.dt.float32
    AF = mybir.ActivationFunctionType
    Alu = mybir.AluOpType

    nc = bass.Bass("TRN2", debug=False)

    feats_leafT = nc.dram_tensor(
        "feats_leafT", [X, NLEAF * BC], fp32, kind="ExternalInput"
    ).ap()
    feats_rootT = nc.dram_tensor("feats_rootT", [X, BC], fp32, kind="ExternalInput").ap()
    wbig_d = nc.dram_tensor("wbig", [128, 12 * 128], fp32, kind="ExternalInput").ap()
    bbig_d = nc.dram_tensor("bbig", [128, 8], fp32, kind="ExternalInput").ap()
    out_d = nc.dram_tensor("out", [NCOUT, BC], fp32, kind="ExternalOutput").ap()

    with tile.TileContext(nc) as tc, ExitStack() as ctx:
        const = ctx.enter_context(tc.tile_pool(name="const", bufs=1))
        feats_pool = ctx.enter_context(tc.tile_pool(name="feats", bufs=3))
        gates = ctx.enter_context(tc.tile_pool(name="gates", bufs=3))
        accp = ctx.enter_context(tc.tile_pool(name="acc", bufs=2))
        psum = ctx.enter_context(tc.tile_pool(name="psum", bufs=1, space="PSUM"))

        wbig = const.tile([128, 12 * 128], fp32, name="wbig_sb")
        nc.sync.dma_start(wbig, wbig_d)
        bbig = const.tile([128, 8], fp32, name="bbig_sb")
        nc.sync.dma_start(bbig, bbig_d)

        def W(i):
            return wbig[:, 128 * i : 128 * (i + 1)]

        w_mlp = W(0)
        w_iou = [W(1), W(2), W(3)]  # leaf W_iou_bu.T slices (i, o, u)
        u_iou = [W(4), W(5), W(6)]  # internal U_iou_bu.T slices
        uf = W(7)
        wtd_i, wtd_u = W(8), W(9)
        wfc_bu, wfc_td = W(10), W(11)

        def bias(i):
            return bbig[:, i : i + 1]

        b_mlp, bi, bo, bu, bf, bi_td, bu_td, b_fc = [bias(i) for i in range(8)]

        # parent level -> (h_full, fc_full) accumulator tiles holding the
        # child level's h and f*c for one parent chunk (width 2*parent_chunk).
        acc = {}

        def get_acc(pl):
            if pl not in acc:
                w2 = 2 * min(_t(pl), CHUNK)
                hf = accp.tile([128, w2], fp32, tag=f"hf{pl}", bufs=2, name=f"hf{pl}")
                ff = accp.tile([128, w2], fp32, tag=f"ff{pl}", bufs=2, name=f"ff{pl}")
                acc[pl] = (hf, ff)
            return acc[pl]

        def evenodd(t, w2):
            # [128, w2] -> even/odd node-block views [128, w2/64, 32]
            v = t[:, 0:w2].rearrange("p (n t b) -> p n t b", t=2, b=BC)
            return v[:, :, 0, :], v[:, :, 1, :]

        croot = {}

        def process_chunk(l, idx, src=None):
            """Process one 512-col chunk of level l (gates + f), writing h and
            f*c into the parent accumulator."""
            w = min(_t(l), CHUNK)
            need_h = l > 0
            iou_ps = psum.tile([128, 3, CHUNK], fp32, tag="iou", bufs=2, name="iou_ps")
            gs = [0, 1, 2] if need_h else [0, 2]
            if l == DEPTH - 1:
                for g in gs:
                    nc.tensor.matmul(iou_ps[:, g, :w], w_iou[g], src, start=True, stop=True)
            else:
                hf, ff = acc.pop(l)
                w2 = 2 * w
                he, ho = evenodd(hf, w2)
                for g in gs:
                    nc.tensor.matmul(iou_ps[:, g, :w], u_iou[g], he, start=True, stop=False)
                    nc.tensor.matmul(iou_ps[:, g, :w], u_iou[g], ho, start=False, stop=True)

            si = gates.tile([128, w], fp32, tag="si", bufs=3, name="si")
            nc.scalar.activation(si, iou_ps[:, 0, :w], AF.Sigmoid, bias=bi)
            tu = gates.tile([128, w], fp32, tag="tu", bufs=3, name="tu")
            nc.scalar.activation(tu, iou_ps[:, 2, :w], AF.Tanh, bias=bu)

            if need_h:
                ct = gates.tile([128, w], fp32, tag="c", bufs=3, name="ct")
            else:
                ct = const.tile([128, w], fp32, name="croot_bu")
            nc.vector.tensor_mul(ct, si, tu)
            if l < DEPTH - 1:
                fe, fo = evenodd(ff, w2)
                ctv = ct[:, 0:w].rearrange("p (n b) -> p n b", b=BC)
                nc.vector.tensor_add(ctv, ctv, fe)
                nc.vector.tensor_add(ctv, ctv, fo)

            if not need_h:
                croot["bu"] = ct
                return

            so = gates.tile([128, w], fp32, tag="so", bufs=3, name="so")
            nc.scalar.activation(so, iou_ps[:, 1, :w], AF.Sigmoid, bias=bo)
            tct = gates.tile([128, w], fp32, tag="tc", bufs=3, name="tct")
            nc.scalar.activation(tct, ct, AF.Tanh)

            pl = l - 1
            hfp, ffp = get_acc(pl)
            w2p = 2 * min(_t(pl), CHUNK)
            off = (idx % (w2p // w)) * w
            hslot = hfp[:, off : off + w]
            nc.vector.tensor_mul(hslot, so, tct)

            zf = psum.tile([128, CHUNK], fp32, tag="zf", bufs=1, name="zf")
            nc.tensor.matmul(zf[:, :w], uf, hslot, start=True, stop=True)
            fg = gates.tile([128, w], fp32, tag="f", bufs=3, name="fg")
            nc.scalar.activation(fg, zf[:, :w], AF.Sigmoid, bias=bf)
            nc.gpsimd.tensor_mul(ffp[:, off : off + w], fg, ct)

        def on_complete(l, idx):
            if l == 0:
                return
            pl = l - 1
            w = min(_t(l), CHUNK)
            pw = min(_t(pl), CHUNK)
            cpp = (2 * pw) // w  # child chunks per parent chunk
            if (idx + 1) % cpp == 0:
                pidx = idx // cpp
                process_chunk(pl, pidx)
                on_complete(pl, pidx)

        # Bottom-up pass, depth-first over leaf chunks.
        for j in range(NLEAF * BC // CHUNK):
            ft = feats_pool.tile([128, CHUNK], fp32, tag="feats", bufs=3, name="ft")
            nc.sync.dma_start(ft, feats_leafT[:, j * CHUNK : (j + 1) * CHUNK])
            mlp_ps = psum.tile([128, CHUNK], fp32, tag="mlp", bufs=1, name="mlp_ps")
            nc.tensor.matmul(mlp_ps, w_mlp, ft, start=True, stop=True)
            xt = gates.tile([128, CHUNK], fp32, tag="x", bufs=3, name="xt")
            nc.vector.tensor_scalar(xt, mlp_ps, b_mlp, 0.0, Alu.add, Alu.max)
            process_chunk(DEPTH - 1, j, src=xt)
            on_complete(DEPTH - 1, j)

        # Top-down root: c_td = sigmoid(i)*tanh(u) on node 0 only.
        ftr = feats_pool.tile([128, BC], fp32, tag="feats", bufs=3, name="ftr")
        nc.sync.dma_start(ftr, feats_rootT)
        mlp2 = psum.tile([128, CHUNK], fp32, tag="mlp", bufs=1, name="mlp2")
        nc.tensor.matmul(mlp2[:, :BC], w_mlp, ftr, start=True, stop=True)
        xr = gates.tile([128, BC], fp32, tag="x", bufs=3, name="xr")
        nc.vector.tensor_scalar(xr, mlp2[:, :BC], b_mlp, 0.0, Alu.add, Alu.max)
        iou_td = psum.tile([128, 3, CHUNK], fp32, tag="iou", bufs=2, name="iou_td")
        nc.tensor.matmul(iou_td[:, 0, :BC], wtd_i, xr, start=True, stop=True)
        nc.tensor.matmul(iou_td[:, 2, :BC], wtd_u, xr, start=True, stop=True)
        si_td = gates.tile([128, BC], fp32, tag="si", bufs=3, name="si_td")
        nc.scalar.activation(si_td, iou_td[:, 0, :BC], AF.Sigmoid, bias=bi_td)
        tu_td = gates.tile([128, BC], fp32, tag="tu", bufs=3, name="tu_td")
        nc.scalar.activation(tu_td, iou_td[:, 2, :BC], AF.Tanh, bias=bu_td)
        c_td = const.tile([128, BC], fp32, name="c_td")
        nc.vector.tensor_mul(c_td, si_td, tu_td)

        # Readout: out = W_fc @ [c_bu_root; c_td_root] + b_fc  (output kept
        # transposed as [NC, BC]; host transposes back).
        fc_ps = psum.tile([128, CHUNK], fp32, tag="mlp", bufs=1, name="fc_ps")
        nc.tensor.matmul(fc_ps[:, :BC], wfc_bu, croot["bu"], start=True, stop=False)
        nc.tensor.matmul(fc_ps[:, :BC], wfc_td, c_td, start=False, stop=True)
        out_sb = gates.tile([128, BC], fp32, tag="si", bufs=3, name="out_sb")
        nc.scalar.activation(out_sb, fc_ps[:, :BC], AF.Identity, bias=b_fc)
        nc.sync.dma_start(out_d, out_sb)

    _split_multi_waits(nc)
    return nc


def _prep_shared(inputs):
    f32 = np.float32

    def T(a):
        return np.ascontiguousarray(np.asarray(a, f32).T)

    W_fc = np.asarray(inputs["W_fc"], f32)
    wbig = np.concatenate(
        [
            T(inputs["W_mlp"]),
            T(inputs["W_iou_bu"]),  # [128, 384] = i|o|u
            T(inputs["U_iou_bu"]),
            T(inputs["Uf_bu_w"]),
            T(np.asarray(inputs["W_iou_td"], f32)[0:128, :]),  # i slice
            T(np.asarray(inputs["W_iou_td"], f32)[256:384, :]),  # u slice
            T(W_fc[:, 0:128]),
            T(W_fc[:, 128:256]),
        ],
        axis=1,
    )
    b_iou_bu = np.asarray(inputs["b_iou_bu"], f32)
    b_iou_td = np.asarray(inputs["b_iou_td"], f32)
    bbig = np.stack(
        [
            np.asarray(inputs["b_mlp"], f32),
            b_iou_bu[0:128],
            b_iou_bu[128:256],
            b_iou_bu[256:384],
            np.asarray(inputs["Uf_bu_b"], f32),
            b_iou_td[0:128],
            b_iou_td[256:384],
            np.asarray(inputs["b_fc"], f32),
        ],
        axis=1,
    )
    return np.ascontiguousarray(wbig), np.ascontiguousarray(bbig)


def _get_runner():
    """Build the bass program once and return a cached jitted 8-core runner.

    Mirrors concourse.bass2jax.run_bass_via_pjrt but caches the jitted
    callable so repeated kernel() calls don't re-trace/re-compile."""
    if "runner" in _CACHE:
        return _CACHE["runner"]

    import jax
    import jax.numpy as jnp
    from jax.sharding import Mesh, PartitionSpec
    from jax.experimental.shard_map import shard_map

    import concourse.mybir as mybir
    from concourse import bass2jax

    bass2jax.install_neuronx_cc_hook()
    nc = _build_nc()

    partition_name = (
        nc.partition_id_tensor.name if nc.partition_id_tensor is not None else None
    )
    in_names, out_names, out_avals = [], [], []
    for alloc in nc.m.functions[0].allocations:
        if not isinstance(alloc, mybir.MemoryLocationSet):
            continue
        name = alloc.memorylocations[0].name
        if alloc.kind == "ExternalInput":
            if name != partition_name:
                in_names.append(name)
        elif alloc.kind == "ExternalOutput":
            out_names.append(name)
            out_avals.append(
                jax.core.ShapedArray(
                    tuple(alloc.tensor_shape), mybir.dt.np(alloc.dtype)
                )
            )
    n_params = len(in_names)
    all_in_names = in_names + out_names
    if partition_name is not None:
        all_in_names = all_in_names + [partition_name]

    def _body(*args):
        operands = list(args)
        if partition_name is not None:
            operands.append(bass2jax.partition_id_tensor())
        outs = bass2jax._bass_exec_p.bind(
            *operands,
            out_avals=tuple(out_avals),
            in_names=tuple(all_in_names),
            out_names=tuple(out_names),
            lowering_input_output_aliases=(),
            sim_require_finite=True,
            sim_require_nnan=True,
            nc=nc,
        )
        return tuple(outs)

    devices = jax.devices()[:NCORES]
    mesh = Mesh(np.asarray(devices), ("core",))
    n_outs = len(out_names)
    sharded = jax.jit(
        shard_map(
            _body,
            mesh=mesh,
            in_specs=(PartitionSpec("core"),) * (n_params + n_outs),
            out_specs=(PartitionSpec("core"),) * n_outs,
            check_rep=False,
        ),
        keep_unused=True,
    )

    runner = {
        "nc": nc,
        "sharded": sharded,
        "in_names": in_names,
        "out_names": out_names,
        "out_avals": out_avals,
        "mesh": mesh,
    }
    _CACHE["runner"] = runner
    return runner


def _run_spmd(in_maps):
    """Execute on 8 cores; returns list of per-core output dicts."""
    r = _get_runner()
    concat_in = [
        np.concatenate([m[name] for m in in_maps], axis=0) for name in r["in_names"]
    ]
    concat_zeros = [
        np.zeros((NCORES * a.shape[0], *a.shape[1:]), a.dtype) for a in r["out_avals"]
    ]
    out_arrs = r["sharded"](*concat_in, *concat_zeros)
    return [
        {
            name: np.asarray(out_arrs[i]).reshape(NCORES, *r["out_avals"][i].shape)[c]
            for i, name in enumerate(r["out_names"])
        }
        for c in range(NCORES)
    ]


def kernel(**inputs):
    global LAST_RESULTS

    feats = np.asarray(inputs["feats"], np.float32)  # [256, 1023, 128]
    wbig, bbig = _prep_shared(inputs)

    in_maps = []
    for c in range(NCORES):
        fb = feats[c * BC : (c + 1) * BC]  # [BC, 1023, 128]
        leafT = np.ascontiguousarray(
            fb[:, NLEAF - 1 : 2 * NLEAF - 1, :].transpose(2, 1, 0).reshape(X, NLEAF * BC)
        )
        rootT = np.ascontiguousarray(fb[:, 0, :].T)
        in_maps.append(
            {
                "feats_leafT": leafT,
                "feats_rootT": rootT,
                "wbig": wbig,
                "bbig": bbig,
            }
        )

    results = _run_spmd(in_maps)
    LAST_RESULTS = results
    out = np.concatenate([results[c]["out"].T for c in range(NCORES)], axis=0)
    return np.ascontiguousarray(out.astype(np.float32))


# revision 10
# speedup vs baseline: 22.1145x; 1.3099x over previous
# Bidirectional TreeLSTM (heap-indexed complete binary tree) on 8 trn2 NeuronCores.
#
# Key algorithmic reductions vs the reference:
#   * The output reads only c_bu[:, 0] and c_td[:, 0].  The entire top-down
#     recurrence below the root is dead code; only the root's W_iou_td path is
#     computed.  x = relu(feats @ W_mlp.T + b) is needed only at the 512 leaf
#     nodes (bottom-up) and at node 0 (top-down root), so only 513/1023 of
#     feats is ever loaded.
#   * Sharding: data-parallel over B (32 trees/core), weights replicated.
#
# Layout per core: tokens are columns (node-major, batch-minor); features on
# the 128 partitions.  Levels processed leaves->root depth-first in chunks of
# 512 columns; each chunk's h / f*c go directly into the parent level's
# accumulator tile, and the parent's iou matmuls read even/odd child blocks
# with PSUM accumulation (h_l + h_r folded into the matmul).

import numpy as np

B, DEPTH, X, H = 256, 10, 128, 128
NCOUT = 128
NCORES = 8
BC = B // NCORES  # trees per core
NLEAF = 512
CHUNK = 512

_CACHE = {}
LAST_RESULTS = None  # BassKernelResults from the most recent run (for test.py)


def _t(level):
    """Tokens (columns) at a tree level, per core."""
    return BC * (1 << level)


def _split_multi_waits(nc):
    """This container's walrus supports only ONE embedded sem-wait per
    instruction.  Hoist extra waits onto same-engine NOPs inserted directly
    before each offending instruction (sem-wait order is immaterial)."""
    import concourse.mybir as mybir

    n_split = 0
    for fn in nc.m.functions:
        for bb in fn.blocks:
            out = []
            changed = False
            for inst in bb.instructions:
                si = inst.sync_info
                if si is not None and len(si.on_wait) > 1:
                    waits = list(si.on_wait)
                    for k, wt in enumerate(waits[:-1]):
                        nop = mybir.InstNoOp(
                            name=f"{inst.name}_wsplit{k}", ins=[], outs=[]
                        )
                        nop.engine = inst.engine
                        nop.sync_info = mybir.SyncInfo(on_wait=[wt], on_update=[])
                        out.append(nop)
                        n_split += 1
                    inst.sync_info = mybir.SyncInfo(
                        on_wait=waits[-1:], on_update=list(si.on_update)
                    )
                    changed = True
                out.append(inst)
            if changed:
                bb.instructions = out
    return n_split


def _build_nc():
    from contextlib import ExitStack

    import concourse.bass as bass
    import concourse.mybir as mybir
    import concourse.tile as tile

    fp32 = mybir.dt.float32
    f32r = mybir.dt.float32r
    AF = mybir.ActivationFunctionType
    Alu = mybir.AluOpType

    nc = bass.Bass("TRN2", debug=False)

    feats_leafT = nc.dram_tensor(
        "feats_leafT", [X, NLEAF * BC], f32r, kind="ExternalInput"
    ).ap()
    feats_rootT = nc.dram_tensor("feats_rootT", [X, BC], f32r, kind="ExternalInput").ap()
    wbig_d = nc.dram_tensor("wbig", [128, 12 * 128], f32r, kind="ExternalInput").ap()
    bbig_d = nc.dram_tensor("bbig", [128, 8], fp32, kind="ExternalInput").ap()
    out_d = nc.dram_tensor("out", [NCOUT, BC], fp32, kind="ExternalOutput").ap()

    with tile.TileContext(nc) as tc, ExitStack() as ctx:
        const = ctx.enter_context(tc.tile_pool(name="const", bufs=1))
        feats_pool = ctx.enter_context(tc.tile_pool(name="feats", bufs=3))
        gates = ctx.enter_context(tc.tile_pool(name="gates", bufs=3))
        accp = ctx.enter_context(tc.tile_pool(name="acc", bufs=2))
        psum = ctx.enter_context(tc.tile_pool(name="psum", bufs=1, space="PSUM"))

        wbig = const.tile([128, 12 * 128], f32r, name="wbig_sb")
        nc.sync.dma_start(wbig, wbig_d)
        bbig = const.tile([128, 8], fp32, name="bbig_sb")
        nc.sync.dma_start(bbig, bbig_d)

        def W(i):
            return wbig[:, 128 * i : 128 * (i + 1)]

        w_mlp = W(0)
        w_iou = [W(1), W(2), W(3)]  # leaf W_iou_bu.T slices (i, o, u)
        u_iou = [W(4), W(5), W(6)]  # internal U_iou_bu.T slices
        uf = W(7)
        wtd_i, wtd_u = W(8), W(9)
        wfc_bu, wfc_td = W(10), W(11)

        def bias(i):
            return bbig[:, i : i + 1]

        b_mlp, bi, bo, bu, bf, bi_td, bu_td, b_fc = [bias(i) for i in range(8)]

        # parent level -> (h_full, fc_full) accumulator tiles holding the
        # child level's h and f*c for one parent chunk (width 2*parent_chunk).
        acc = {}

        def get_acc(pl):
            if pl not in acc:
                w2 = 2 * min(_t(pl), CHUNK)
                hf = accp.tile([128, w2], f32r, tag=f"hf{pl}", bufs=2, name=f"hf{pl}")
                ff = accp.tile([128, w2], fp32, tag=f"ff{pl}", bufs=2, name=f"ff{pl}")
                acc[pl] = (hf, ff)
            return acc[pl]

        def evenodd(t, w2):
            # [128, w2] -> even/odd node-block views [128, w2/64, 32]
            v = t[:, 0:w2].rearrange("p (n t b) -> p n t b", t=2, b=BC)
            return v[:, :, 0, :], v[:, :, 1, :]

        croot = {}

        def process_chunk(l, idx, src=None):
            """Process one 512-col chunk of level l (gates + f), writing h and
            f*c into the parent accumulator."""
            w = min(_t(l), CHUNK)
            need_h = l > 0
            iou_ps = psum.tile([128, 3, CHUNK], fp32, tag="iou", bufs=2, name="iou_ps")
            gs = [0, 1, 2] if need_h else [0, 2]
            if l == DEPTH - 1:
                for g in gs:
                    nc.tensor.matmul(iou_ps[:, g, :w], w_iou[g], src, start=True, stop=True)
            else:
                hf, ff = acc.pop(l)
                w2 = 2 * w
                he, ho = evenodd(hf, w2)
                for g in gs:
                    nc.tensor.matmul(iou_ps[:, g, :w], u_iou[g], he, start=True, stop=False)
                    nc.tensor.matmul(iou_ps[:, g, :w], u_iou[g], ho, start=False, stop=True)

            si = gates.tile([128, w], fp32, tag="si", bufs=3, name="si")
            nc.scalar.activation(si, iou_ps[:, 0, :w], AF.Sigmoid, bias=bi)
            tu = gates.tile([128, w], fp32, tag="tu", bufs=3, name="tu")
            nc.scalar.activation(tu, iou_ps[:, 2, :w], AF.Tanh, bias=bu)

            if need_h:
                ct = gates.tile([128, w], fp32, tag="c", bufs=3, name="ct")
            else:
                ct = const.tile([128, w], f32r, name="croot_bu")
            nc.vector.tensor_mul(ct, si, tu)
            if l < DEPTH - 1:
                fe, fo = evenodd(ff, w2)
                ctv = ct[:, 0:w].rearrange("p (n b) -> p n b", b=BC)
                nc.vector.tensor_add(ctv, ctv, fe)
                nc.vector.tensor_add(ctv, ctv, fo)

            if not need_h:
                croot["bu"] = ct
                return

            so = gates.tile([128, w], fp32, tag="so", bufs=3, name="so")
            nc.scalar.activation(so, iou_ps[:, 1, :w], AF.Sigmoid, bias=bo)
            tct = gates.tile([128, w], fp32, tag="tc", bufs=3, name="tct")
            nc.scalar.activation(tct, ct, AF.Tanh)

            pl = l - 1
            hfp, ffp = get_acc(pl)
            w2p = 2 * min(_t(pl), CHUNK)
            off = (idx % (w2p // w)) * w
            hslot = hfp[:, off : off + w]
            nc.vector.tensor_mul(hslot, so, tct)

            zf = psum.tile([128, CHUNK], fp32, tag="zf", bufs=1, name="zf")
            nc.tensor.matmul(zf[:, :w], uf, hslot, start=True, stop=True)
            fg = gates.tile([128, w], fp32, tag="f", bufs=3, name="fg")
            nc.scalar.activation(fg, zf[:, :w], AF.Sigmoid, bias=bf)
            nc.gpsimd.tensor_mul(ffp[:, off : off + w], fg, ct)

        def on_complete(l, idx):
            if l == 0:
                return
            pl = l - 1
            w = min(_t(l), CHUNK)
            pw = min(_t(pl), CHUNK)
            cpp = (2 * pw) // w  # child chunks per parent chunk
            if (idx + 1) % cpp == 0:
                pidx = idx // cpp
                process_chunk(pl, pidx)
                on_complete(pl, pidx)

        # Bottom-up pass, depth-first over leaf chunks.
        for j in range(NLEAF * BC // CHUNK):
            ft = feats_pool.tile([128, CHUNK], f32r, tag="feats", bufs=3, name="ft")
            nc.sync.dma_start(ft, feats_leafT[:, j * CHUNK : (j + 1) * CHUNK])
            mlp_ps = psum.tile([128, CHUNK], fp32, tag="mlp", bufs=1, name="mlp_ps")
            nc.tensor.matmul(mlp_ps, w_mlp, ft, start=True, stop=True)
            xt = gates.tile([128, CHUNK], f32r, tag="x", bufs=3, name="xt")
            nc.vector.tensor_scalar(xt, mlp_ps, b_mlp, 0.0, Alu.add, Alu.max)
            process_chunk(DEPTH - 1, j, src=xt)
            on_complete(DEPTH - 1, j)

        # Top-down root: c_td = sigmoid(i)*tanh(u) on node 0 only.
        ftr = feats_pool.tile([128, BC], f32r, tag="feats", bufs=3, name="ftr")
        nc.sync.dma_start(ftr, feats_rootT)
        mlp2 = psum.tile([128, CHUNK], fp32, tag="mlp", bufs=1, name="mlp2")
        nc.tensor.matmul(mlp2[:, :BC], w_mlp, ftr, start=True, stop=True)
        xr = gates.tile([128, BC], f32r, tag="x", bufs=3, name="xr")
        nc.vector.tensor_scalar(xr, mlp2[:, :BC], b_mlp, 0.0, Alu.add, Alu.max)
        iou_td = psum.tile([128, 3, CHUNK], fp32, tag="iou", bufs=2, name="iou_td")
        nc.tensor.matmul(iou_td[:, 0, :BC], wtd_i, xr, start=True, stop=True)
        nc.tensor.matmul(iou_td[:, 2, :BC], wtd_u, xr, start=True, stop=True)
        si_td = gates.tile([128, BC], fp32, tag="si", bufs=3, name="si_td")
        nc.scalar.activation(si_td, iou_td[:, 0, :BC], AF.Sigmoid, bias=bi_td)
        tu_td = gates.tile([128, BC], fp32, tag="tu", bufs=3, name="tu_td")
        nc.scalar.activation(tu_td, iou_td[:, 2, :BC], AF.Tanh, bias=bu_td)
        c_td = const.tile([128, BC], f32r, name="c_td")
        nc.vector.tensor_mul(c_td, si_td, tu_td)

        # Readout: out = W_fc @ [c_bu_root; c_td_root] + b_fc  (output kept
        # transposed as [NC, BC]; host transposes back).
        fc_ps = psum.tile([128, CHUNK], fp32, tag="mlp", bufs=1, name="fc_ps")
        nc.tensor.matmul(fc_ps[:, :BC], wfc_bu, croot["bu"], start=True, stop=False)
        nc.tensor.matmul(fc_ps[:, :BC], wfc_td, c_td, start=False, stop=True)
        out_sb = gates.tile([128, BC], fp32, tag="si", bufs=3, name="out_sb")
        nc.scalar.activation(out_sb, fc_ps[:, :BC], AF.Identity, bias=b_fc)
        nc.sync.dma_start(out_d, out_sb)

    _split_multi_waits(nc)
    return nc


def _prep_shared(inputs):
    f32 = np.float32

    def T(a):
        return np.ascontiguousarray(np.asarray(a, f32).T)

    W_fc = np.asarray(inputs["W_fc"], f32)
    wbig = np.concatenate(
        [
            T(inputs["W_mlp"]),
            T(inputs["W_iou_bu"]),  # [128, 384] = i|o|u
            T(inputs["U_iou_bu"]),
            T(inputs["Uf_bu_w"]),
            T(np.asarray(inputs["W_iou_td"], f32)[0:128, :]),  # i slice
            T(np.asarray(inputs["W_iou_td"], f32)[256:384, :]),  # u slice
            T(W_fc[:, 0:128]),
            T(W_fc[:, 128:256]),
        ],
        axis=1,
    )
    b_iou_bu = np.asarray(inputs["b_iou_bu"], f32)
    b_iou_td = np.asarray(inputs["b_iou_td"], f32)
    bbig = np.stack(
        [
            np.asarray(inputs["b_mlp"], f32),
            b_iou_bu[0:128],
            b_iou_bu[128:256],
            b_iou_bu[256:384],
            np.asarray(inputs["Uf_bu_b"], f32),
            b_iou_td[0:128],
            b_iou_td[256:384],
            np.asarray(inputs["b_fc"], f32),
        ],
        axis=1,
    )
    return np.ascontiguousarray(wbig), np.ascontiguousarray(bbig)


def _get_runner():
    """Build the bass program once and return a cached jitted 8-core runner.

    Mirrors concourse.bass2jax.run_bass_via_pjrt but caches the jitted
    callable so repeated kernel() calls don't re-trace/re-compile."""
    if "runner" in _CACHE:
        return _CACHE["runner"]

    import jax
    import jax.numpy as jnp
    from jax.sharding import Mesh, PartitionSpec
    from jax.experimental.shard_map import shard_map

    import concourse.mybir as mybir
    from concourse import bass2jax

    bass2jax.install_neuronx_cc_hook()
    nc = _build_nc()

    partition_name = (
        nc.partition_id_tensor.name if nc.partition_id_tensor is not None else None
    )
    in_names, out_names, out_avals = [], [], []
    for alloc in nc.m.functions[0].allocations:
        if not isinstance(alloc, mybir.MemoryLocationSet):
            continue
        name = alloc.memorylocations[0].name
        if alloc.kind == "ExternalInput":
            if name != partition_name:
                in_names.append(name)
        elif alloc.kind == "ExternalOutput":
            out_names.append(name)
            out_avals.append(
                jax.core.ShapedArray(
                    tuple(alloc.tensor_shape), mybir.dt.np(alloc.dtype)
                )
            )
    n_params = len(in_names)
    all_in_names = in_names + out_names
    if partition_name is not None:
        all_in_names = all_in_names + [partition_name]

    def _body(*args):
        operands = list(args)
        if partition_name is not None:
            operands.append(bass2jax.partition_id_tensor())
        outs = bass2jax._bass_exec_p.bind(
            *operands,
            out_avals=tuple(out_avals),
            in_names=tuple(all_in_names),
            out_names=tuple(out_names),
            lowering_input_output_aliases=(),
            sim_require_finite=True,
            sim_require_nnan=True,
            nc=nc,
        )
        return tuple(outs)

    devices = jax.devices()[:NCORES]
    mesh = Mesh(np.asarray(devices), ("core",))
    n_outs = len(out_names)
    sharded = jax.jit(
        shard_map(
            _body,
            mesh=mesh,
            in_specs=(PartitionSpec("core"),) * (n_params + n_outs),
            out_specs=(PartitionSpec("core"),) * n_outs,
            check_rep=False,
        ),
        keep_unused=True,
    )

    runner = {
        "nc": nc,
        "sharded": sharded,
        "in_names": in_names,
        "out_names": out_names,
        "out_avals": out_avals,
        "mesh": mesh,
    }
    _CACHE["runner"] = runner
    return runner


def _run_spmd(in_maps):
    """Execute on 8 cores; returns list of per-core output dicts."""
    r = _get_runner()
    concat_in = [
        np.concatenate([m[name] for m in in_maps], axis=0) for name in r["in_names"]
    ]
    concat_zeros = [
        np.zeros((NCORES * a.shape[0], *a.shape[1:]), a.dtype) for a in r["out_avals"]
    ]
    out_arrs = r["sharded"](*concat_in, *concat_zeros)
    return [
        {
            name: np.asarray(out_arrs[i]).reshape(NCORES, *r["out_avals"][i].shape)[c]
            for i, name in enumerate(r["out_names"])
        }
        for c in range(NCORES)
    ]


def kernel(**inputs):
    global LAST_RESULTS

    feats = np.asarray(inputs["feats"], np.float32)  # [256, 1023, 128]
    wbig, bbig = _prep_shared(inputs)

    in_maps = []
    for c in range(NCORES):
        fb = feats[c * BC : (c + 1) * BC]  # [BC, 1023, 128]
        leafT = np.ascontiguousarray(
            fb[:, NLEAF - 1 : 2 * NLEAF - 1, :].transpose(2, 1, 0).reshape(X, NLEAF * BC)
        )
        rootT = np.ascontiguousarray(fb[:, 0, :].T)
        in_maps.append(
            {
                "feats_leafT": leafT,
                "feats_rootT": rootT,
                "wbig": wbig,
                "bbig": bbig,
            }
        )

    results = _run_spmd(in_maps)
    LAST_RESULTS = results
    out = np.concatenate([results[c]["out"].T for c in range(NCORES)], axis=0)
    return np.ascontiguousarray(out.astype(np.float32))


# revision 27
# speedup vs baseline: 9981.6008x; 451.3596x over previous
# Bidirectional TreeLSTM (heap-indexed complete binary tree) on 8 trn2 NeuronCores.
#
# Key algorithmic reductions vs the reference:
#   * The output reads only c_bu[:, 0] and c_td[:, 0].  The entire top-down
#     recurrence below the root is dead code; only the root's W_iou_td path is
#     computed.  x = relu(feats @ W_mlp.T + b) is needed only at the 512 leaf
#     nodes (bottom-up) and at node 0 (top-down root), so only 513/1023 of
#     feats is ever loaded.
#   * Sharding: data-parallel over B (32 trees/core), weights replicated.
#
# Layout per core: tokens are columns (node-major, batch-minor); features on
# the 128 partitions.  Levels processed leaves->root depth-first in chunks of
# 512 columns; each chunk's h / f*c go directly into the parent level's
# accumulator tile, and the parent's iou matmuls read even/odd child blocks
# with PSUM accumulation (h_l + h_r folded into the matmul).

import numpy as np

B, DEPTH, X, H = 256, 10, 128, 128
NCOUT = 128
NCORES = 8
BC = B // NCORES  # trees per core
NLEAF = 512
CHUNK = 512

_CACHE = {}
LAST_RESULTS = None  # BassKernelResults from the most recent run (for test.py)


def _t(level):
    """Tokens (columns) at a tree level, per core."""
    return BC * (1 << level)


def _split_multi_waits(nc):
    """This container's walrus supports only ONE embedded sem-wait per
    instruction.  Hoist extra waits onto same-engine NOPs inserted directly
    before each offending instruction (sem-wait order is immaterial)."""
    import concourse.mybir as mybir

    n_split = 0
    for fn in nc.m.functions:
        for bb in fn.blocks:
            out = []
            changed = False
            for inst in bb.instructions:
                si = inst.sync_info
                if si is not None and len(si.on_wait) > 1:
                    waits = list(si.on_wait)
                    for k, wt in enumerate(waits[:-1]):
                        nop = mybir.InstNoOp(
                            name=f"{inst.name}_wsplit{k}", ins=[], outs=[]
                        )
                        nop.engine = inst.engine
                        nop.sync_info = mybir.SyncInfo(on_wait=[wt], on_update=[])
                        out.append(nop)
                        n_split += 1
                    inst.sync_info = mybir.SyncInfo(
                        on_wait=waits[-1:], on_update=list(si.on_update)
                    )
                    changed = True
                out.append(inst)
            if changed:
                bb.instructions = out
    return n_split


def _build_nc(merge_sio=False, reps=1):
    from contextlib import ExitStack

    import concourse.bass as bass
    import concourse.mybir as mybir
    import concourse.tile as tile

    fp32 = mybir.dt.float32
    f32r = mybir.dt.float32r
    AF = mybir.ActivationFunctionType
    Alu = mybir.AluOpType

    nc = bass.Bass("TRN2", debug=False)

    feats_leafT = nc.dram_tensor(
        "feats_leafT", [X, NLEAF * BC], f32r, kind="ExternalInput"
    ).ap()
    feats_rootT = nc.dram_tensor("feats_rootT", [X, BC], f32r, kind="ExternalInput").ap()
    wbig_d = nc.dram_tensor("wbig", [128, 12 * 128], f32r, kind="ExternalInput").ap()
    bbig_d = nc.dram_tensor("bbig", [128, 8], fp32, kind="ExternalInput").ap()
    out_d = nc.dram_tensor("out", [NCOUT, BC], fp32, kind="ExternalOutput").ap()

    with tile.TileContext(nc) as tc, ExitStack() as ctx:
        const = ctx.enter_context(tc.tile_pool(name="const", bufs=1))
        feats_pool = ctx.enter_context(tc.tile_pool(name="feats", bufs=3))
        gates = ctx.enter_context(tc.tile_pool(name="gates", bufs=3))
        accp = ctx.enter_context(tc.tile_pool(name="acc", bufs=2))
        psum = ctx.enter_context(tc.tile_pool(name="psum", bufs=1, space="PSUM"))

        wbig = const.tile([128, 12 * 128], f32r, name="wbig_sb")
        nc.sync.dma_start(wbig, wbig_d)
        bbig = const.tile([128, 8], fp32, name="bbig_sb")
        nc.sync.dma_start(bbig, bbig_d)

        def W(i):
            return wbig[:, 128 * i : 128 * (i + 1)]

        w_mlp = W(0)
        w_iou = [W(1), W(2), W(3)]  # leaf W_iou_bu.T slices (i, o, u)
        u_iou = [W(4), W(5), W(6)]  # internal U_iou_bu.T slices
        uf = W(7)
        wtd_i, wtd_u = W(8), W(9)
        wfc_bu, wfc_td = W(10), W(11)

        def bias(i):
            return bbig[:, i : i + 1]

        b_mlp, bi, bo, bu, bf, bi_td, bu_td, b_fc = [bias(i) for i in range(8)]

        # Accumulator tiles holding child-level h and f*c spans.  Widths are
        # stripe-sized at the top of the tree (so many independent chunks are
        # in flight) and per-parent-chunk at the bottom.
        ACC_W = {8: 4096, 7: 2048, 6: 1024, 5: 1024, 4: 1024, 3: 512, 2: 256, 1: 128, 0: 64}
        acc = {}

        def get_acc(pl, span):
            key = (pl, span)
            if key not in acc:
                w2 = ACC_W[pl]
                nb = 2 if pl >= 6 else 1
                hf = accp.tile([128, w2], f32r, tag=f"hf{pl}", bufs=nb, name=f"hf{pl}")
                ff = accp.tile([128, w2], fp32, tag=f"ff{pl}", bufs=nb, name=f"ff{pl}")
                acc[key] = (hf, ff)
            return acc[key]

        def evenodd(t, off, w2):
            # [128, w2] node-pair view at offset -> even/odd block views
            v = t[:, off : off + w2].rearrange("p (n t b) -> p n t b", t=2, b=BC)
            return v[:, :, 0, :], v[:, :, 1, :]

        croot = {}

        def process_chunk(l, idx, src=None):
            """Process one 512-col chunk of level l (gates + f), writing h and
            f*c into the parent accumulator."""
            w = min(_t(l), CHUNK)
            need_h = l > 0
            iou_ps = psum.tile([128, 3, CHUNK], fp32, tag="iou", bufs=2, name="iou_ps")
            gs = [0, 1, 2] if need_h else [0, 2]
            if l == DEPTH - 1:
                # mlp lands in the i-slot; relu reads it out; the i-matmul
                # then overwrites the same bank (WAR tracked by Tile).
                nc.tensor.matmul(iou_ps[:, 0, :w], w_mlp, src, start=True, stop=True)
                xt = gates.tile([128, CHUNK], f32r, tag="x", bufs=3, name="xt")
                nc.vector.tensor_scalar(xt[:, :w], iou_ps[:, 0, :w], b_mlp, 0.0, Alu.add, Alu.max)
                for g in gs:
                    nc.tensor.matmul(iou_ps[:, g, :w], w_iou[g], xt[:, :w], start=True, stop=True)
            else:
                w2 = 2 * w
                ppa = ACC_W[l] // w2  # parent chunks per acc tile
                hf, ff = get_acc(l, idx // ppa)
                roff = (idx % ppa) * w2
                he, ho = evenodd(hf, roff, w2)
                for g in gs:
                    nc.tensor.matmul(iou_ps[:, g, :w], u_iou[g], he, start=True, stop=False)
                    nc.tensor.matmul(iou_ps[:, g, :w], u_iou[g], ho, start=False, stop=True)
                if idx % ppa == ppa - 1:
                    del acc[(l, idx // ppa)]  # span consumed; pool slot recycles

            if merge_sio and need_h and w == CHUNK:
                sio = gates.tile([128, 2, CHUNK], fp32, tag="sio", bufs=3, name="sio")
                nc.scalar.activation(sio, iou_ps[:, 0:2, :], AF.Sigmoid, bias=bi)
                si, so = sio[:, 0, :w], sio[:, 1, :w]
            else:
                si = gates.tile([128, w], fp32, tag="si", bufs=3, name="si")
                nc.scalar.activation(si, iou_ps[:, 0, :w], AF.Sigmoid, bias=bi)
                so = None
            tu = gates.tile([128, w], fp32, tag="tu", bufs=3, name="tu")
            nc.scalar.activation(tu, iou_ps[:, 2, :w], AF.Tanh, bias=bu)

            if need_h:
                ct = gates.tile([128, w], fp32, tag="c", bufs=3, name="ct")
            else:
                ct = const.tile([128, w], f32r, name="croot_bu")
            nc.vector.tensor_mul(ct, si, tu)
            if l < DEPTH - 1:
                fe, fo = evenodd(ff, roff, w2)
                ctv = ct[:, 0:w].rearrange("p (n b) -> p n b", b=BC)
                nc.vector.tensor_add(ctv, ctv, fe)
                nc.vector.tensor_add(ctv, ctv, fo)

            if not need_h:
                croot["bu"] = ct
                return

            if so is None:
                so = gates.tile([128, w], fp32, tag="so", bufs=3, name="so")
                nc.scalar.activation(so, iou_ps[:, 1, :w], AF.Sigmoid, bias=bo)
            tct = gates.tile([128, w], fp32, tag="tc", bufs=3, name="tct")
            nc.scalar.activation(tct, ct, AF.Tanh)

            pl = l - 1
            cpa = ACC_W[pl] // w  # child chunks per parent-acc tile
            hfp, ffp = get_acc(pl, idx // cpa)
            off = (idx % cpa) * w
            hslot = hfp[:, off : off + w]
            nc.vector.tensor_mul(hslot, so, tct)

            zf = psum.tile([128, CHUNK], fp32, tag="zf", bufs=2, name="zf")
            nc.tensor.matmul(zf[:, :w], uf, hslot, start=True, stop=True)
            fg = gates.tile([128, w], fp32, tag="f", bufs=3, name="fg")
            nc.scalar.activation(fg, zf[:, :w], AF.Sigmoid, bias=bf)
            nc.gpsimd.tensor_mul(ffp[:, off : off + w], fg, ct)

        def on_complete(l, idx):
            if l == 0:
                return
            pl = l - 1
            w = min(_t(l), CHUNK)
            pw = min(_t(pl), CHUNK)
            cpp = (2 * pw) // w  # child chunks per parent chunk
            if (idx + 1) % cpp == 0:
                pidx = idx // cpp
                process_chunk(pl, pidx)
                on_complete(pl, pidx)

        def leaf_chunk(jj):
            ft = feats_pool.tile([128, CHUNK], f32r, tag="feats", bufs=6, name="ft")
            nc.sync.dma_start(ft, feats_leafT[:, jj * CHUNK : (jj + 1) * CHUNK])
            process_chunk(DEPTH - 1, jj, src=ft)

        def one_pass():
            # Top-down root early: independent work to fill the pipeline ramp.
            ftr = feats_pool.tile([128, BC], f32r, tag="feats", bufs=6, name="ftr")
            nc.sync.dma_start(ftr, feats_rootT)
            iou_td = psum.tile([128, 3, CHUNK], fp32, tag="iou", bufs=2, name="iou_td")
            nc.tensor.matmul(iou_td[:, 0, :BC], w_mlp, ftr, start=True, stop=True)
            xr = gates.tile([128, BC], f32r, tag="x", bufs=3, name="xr")
            nc.vector.tensor_scalar(xr, iou_td[:, 0, :BC], b_mlp, 0.0, Alu.add, Alu.max)
            nc.tensor.matmul(iou_td[:, 0, :BC], wtd_i, xr, start=True, stop=True)
            nc.tensor.matmul(iou_td[:, 2, :BC], wtd_u, xr, start=True, stop=True)
            si_td = gates.tile([128, BC], fp32, tag="si", bufs=3, name="si_td")
            nc.scalar.activation(si_td, iou_td[:, 0, :BC], AF.Sigmoid, bias=bi_td)
            tu_td = gates.tile([128, BC], fp32, tag="tu", bufs=3, name="tu_td")
            nc.scalar.activation(tu_td, iou_td[:, 2, :BC], AF.Tanh, bias=bu_td)
            c_td = const.tile([128, BC], f32r, name="c_td")
            nc.vector.tensor_mul(c_td, si_td, tu_td)

            # Software-pipelined stripes, finely interleaved: L8 chunks run
            # one stripe behind the leaves, L7/L6/L5 two stripes behind, so
            # the scalar engine always has independent chunks available.
            def leaf_pair(st, p):
                for j in (2 * p, 2 * p + 1):
                    leaf_chunk(8 * st + j)

            def tail_internals(st):
                for k in range(2):
                    process_chunk(7, 2 * st + k)
                process_chunk(6, st)
                if st % 2 == 1:
                    process_chunk(5, st // 2)
                    on_complete(5, st // 2)

            for s in range(6):
                for p in range(4):
                    if s < 4:
                        leaf_pair(s, p)
                    if 1 <= s <= 4:
                        process_chunk(8, 4 * (s - 1) + p)
                    if s >= 2 and p == 3:
                        tail_internals(s - 2)

            # Readout: out = W_fc @ [c_bu_root; c_td_root] + b_fc  (output
            # kept transposed as [NC, BC]; host transposes back).
            fc_ps = psum.tile([128, CHUNK], fp32, tag="zf", bufs=2, name="fc_ps")
            nc.tensor.matmul(fc_ps[:, :BC], wfc_bu, croot["bu"], start=True, stop=False)
            nc.tensor.matmul(fc_ps[:, :BC], wfc_td, c_td, start=False, stop=True)
            out_sb = gates.tile([128, BC], fp32, tag="si", bufs=3, name="out_sb")
            nc.scalar.activation(out_sb, fc_ps[:, :BC], AF.Identity, bias=b_fc)
            nc.sync.dma_start(out_d, out_sb)

        for _rep in range(reps):
            one_pass()

    _split_multi_waits(nc)
    return nc


def _prep_shared(inputs):
    f32 = np.float32

    def T(a):
        return np.ascontiguousarray(np.asarray(a, f32).T)

    W_fc = np.asarray(inputs["W_fc"], f32)
    wbig = np.concatenate(
        [
            T(inputs["W_mlp"]),
            T(inputs["W_iou_bu"]),  # [128, 384] = i|o|u
            T(inputs["U_iou_bu"]),
            T(inputs["Uf_bu_w"]),
            T(np.asarray(inputs["W_iou_td"], f32)[0:128, :]),  # i slice
            T(np.asarray(inputs["W_iou_td"], f32)[256:384, :]),  # u slice
            T(W_fc[:, 0:128]),
            T(W_fc[:, 128:256]),
        ],
        axis=1,
    )
    b_iou_bu = np.asarray(inputs["b_iou_bu"], f32)
    b_iou_td = np.asarray(inputs["b_iou_td"], f32)
    bbig = np.stack(
        [
            np.asarray(inputs["b_mlp"], f32),
            b_iou_bu[0:128],
            b_iou_bu[128:256],
            b_iou_bu[256:384],
            np.asarray(inputs["Uf_bu_b"], f32),
            b_iou_td[0:128],
            b_iou_td[256:384],
            np.asarray(inputs["b_fc"], f32),
        ],
        axis=1,
    )
    return np.ascontiguousarray(wbig), np.ascontiguousarray(bbig)


def _get_runner(merge_sio=False, reps=1):
    """Build the bass program once and return a cached jitted 8-core runner.

    Mirrors concourse.bass2jax.run_bass_via_pjrt but caches the jitted
    callable so repeated kernel() calls don't re-trace/re-compile."""
    key = ("runner", merge_sio, reps)
    if key in _CACHE:
        return _CACHE[key]

    import jax
    import jax.numpy as jnp
    from jax.sharding import Mesh, PartitionSpec
    from jax.experimental.shard_map import shard_map

    import concourse.mybir as mybir
    from concourse import bass2jax

    bass2jax.install_neuronx_cc_hook()
    nc = _build_nc(merge_sio=merge_sio, reps=reps)

    partition_name = (
        nc.partition_id_tensor.name if nc.partition_id_tensor is not None else None
    )
    in_names, out_names, out_avals = [], [], []
    for alloc in nc.m.functions[0].allocations:
        if not isinstance(alloc, mybir.MemoryLocationSet):
            continue
        name = alloc.memorylocations[0].name
        if alloc.kind == "ExternalInput":
            if name != partition_name:
                in_names.append(name)
        elif alloc.kind == "ExternalOutput":
            out_names.append(name)
            out_avals.append(
                jax.core.ShapedArray(
                    tuple(alloc.tensor_shape), mybir.dt.np(alloc.dtype)
                )
            )
    n_params = len(in_names)
    all_in_names = in_names + out_names
    if partition_name is not None:
        all_in_names = all_in_names + [partition_name]

    def _body(*args):
        operands = list(args)
        if partition_name is not None:
            operands.append(bass2jax.partition_id_tensor())
        outs = bass2jax._bass_exec_p.bind(
            *operands,
            out_avals=tuple(out_avals),
            in_names=tuple(all_in_names),
            out_names=tuple(out_names),
            lowering_input_output_aliases=(),
            sim_require_finite=True,
            sim_require_nnan=True,
            nc=nc,
        )
        return tuple(outs)

    devices = jax.devices()[:NCORES]
    mesh = Mesh(np.asarray(devices), ("core",))
    n_outs = len(out_names)
    sharded = jax.jit(
        shard_map(
            _body,
            mesh=mesh,
            in_specs=(PartitionSpec("core"),) * (n_params + n_outs),
            out_specs=(PartitionSpec("core"),) * n_outs,
            check_rep=False,
        ),
        keep_unused=True,
    )

    runner = {
        "nc": nc,
        "sharded": sharded,
        "in_names": in_names,
        "out_names": out_names,
        "out_avals": out_avals,
        "mesh": mesh,
    }
    _CACHE[key] = runner
    return runner


def _run_spmd(in_maps, merge_sio=False, reps=1):
    """Execute on 8 cores; returns list of per-core output dicts."""
    r = _get_runner(merge_sio, reps)
    concat_in = [
        np.concatenate([m[name] for m in in_maps], axis=0) for name in r["in_names"]
    ]
    concat_zeros = [
        np.zeros((NCORES * a.shape[0], *a.shape[1:]), a.dtype) for a in r["out_avals"]
    ]
    out_arrs = r["sharded"](*concat_in, *concat_zeros)
    return [
        {
            name: np.asarray(out_arrs[i]).reshape(NCORES, *r["out_avals"][i].shape)[c]
            for i, name in enumerate(r["out_names"])
        }
        for c in range(NCORES)
    ]


def kernel(**inputs):
    global LAST_RESULTS

    feats = np.asarray(inputs["feats"], np.float32)  # [256, 1023, 128]
    wbig, bbig = _prep_shared(inputs)
    b_iou_bu = np.asarray(inputs["b_iou_bu"], np.float32)
    merge_sio = bool(np.array_equal(b_iou_bu[0:128], b_iou_bu[128:256]))

    in_maps = []
    for c in range(NCORES):
        fb = feats[c * BC : (c + 1) * BC]  # [BC, 1023, 128]
        leafT = np.ascontiguousarray(
            fb[:, NLEAF - 1 : 2 * NLEAF - 1, :].transpose(2, 1, 0).reshape(X, NLEAF * BC)
        )
        rootT = np.ascontiguousarray(fb[:, 0, :].T)
        in_maps.append(
            {
                "feats_leafT": leafT,
                "feats_rootT": rootT,
                "wbig": wbig,
                "bbig": bbig,
            }
        )

    results = _run_spmd(in_maps, merge_sio=merge_sio)
    LAST_RESULTS = results
    out = np.concatenate([results[c]["out"].T for c in range(NCORES)], axis=0)
    return np.ascontiguousarray(out.astype(np.float32))
